# revision 37
# baseline (speedup 1.0000x reference)
"""TRN2 Bass/Tile kernel for nn_Block_89842125898023 (dense transformer
block), SPMD over 8 NeuronCores.

Sharding (data-parallel over batch x query-halves, zero collectives):
core c handles batch element b = c//2 and query half p = c%2 of that
element's 2048 tokens, using a "zigzag" split (p=0: tokens [0,512) u
[1536,2048); p=1: [512,1536)) so the causal-attention work is identical
on every core. Each core redundantly computes K/V for its batch
element's full sequence from the (replicated) xT input — cheaper than
any cross-core collective.

On-device layout is feature-major ([C, T], channels on partitions).
Q/K/attn stay SBUF-resident (no DRAM round-trips); attention computes
transposed scores S^T[s, t] per head, the softmax normalizer Z comes
from a ones-column appended to V (M=65 AV matmul), the causal mask is
applied multiplicatively post-exp from per-core band-mask inputs, and
the division by Z is deferred to the AV eviction. All matmuls run in
bf16 with fp32 PSUM accumulation; proj and the FFN iterate weights-
outer over the full 1024-token query block so every weight byte is
read from DRAM exactly once. LayerNorm statistics are computed with
ones-vector matmuls on the PE; 1/sqrt(var+eps) is computed as
exp(-0.5*ln(var+eps)) so the whole kernel uses a single activation
table (no LoadActFuncSet churn).

kernel(**inputs) takes the full unsharded inputs, builds per-core input
maps host-side, runs the SPMD program on cores 0-7 via
bass_utils.run_bass_kernel_spmd, and reassembles the full output.
"""

import sys
import os

sys.path.insert(0, "/opt/trn_rl_repo")

from contextlib import ExitStack

import numpy as np
import ml_dtypes

import concourse.bass as bass
import concourse.bacc as bacc
import concourse.tile as tile
from concourse import mybir
from concourse.bass_utils import run_bass_kernel_spmd

F32 = mybir.dt.float32
F32R = mybir.dt.float32r
BF16 = mybir.dt.bfloat16
AF = mybir.ActivationFunctionType
ALU = mybir.AluOpType
P = 128


class Cfg:
    def __init__(self, C=1024, H=16, D=64, Tkv=2048, eps=1e-5, ffn_mult=4):
        self.C = C
        self.H = H
        self.D = D
        assert H * D == C
        self.Tkv = Tkv
        self.Tq = Tkv // 2
        self.F = ffn_mult * C
        self.eps = eps
        self.NC = C // 128
        self.NF = self.F // 128
        self.NS = Tkv // 128
        self.scale = C ** -0.5
        self.TH = self.Tq // 2
        NS2 = self.NS // 2
        self.MB = 128 * (NS2 - 1) + self.TH
        self.MLO = ((Tkv - 128) - 128 * (NS2 - 1), 0)


def build_kernel(nc: bass.Bass, cfg: Cfg, ln_affine=True):
    c = cfg
    NH = c.C // 64

    xT_d = nc.dram_tensor("xT", [c.C, c.Tkv], BF16, kind="ExternalInput")
    xqT_d = nc.dram_tensor("xqT", [c.C, c.Tq], F32R, kind="ExternalInput")
    xqTb_d = nc.dram_tensor("xqTb", [c.C, c.Tq], BF16, kind="ExternalInput")
    wq_d = nc.dram_tensor("wq", [c.C, c.C], BF16, kind="ExternalInput")
    wk_d = nc.dram_tensor("wk", [c.C, c.C], BF16, kind="ExternalInput")
    wv_d = nc.dram_tensor("wv", [c.C, c.C], BF16, kind="ExternalInput")
    wp_d = nc.dram_tensor("wp", [c.C, c.C], BF16, kind="ExternalInput")
    w1_d = nc.dram_tensor("w1", [c.C, c.F], BF16, kind="ExternalInput")
    w2_d = nc.dram_tensor("w2", [c.F, c.C], BF16, kind="ExternalInput")
    NV = 3 * (c.C // P) + c.F // P
    vecs_d = nc.dram_tensor("vecs", [P, NV], F32, kind="ExternalInput")
    mask_d = [nc.dram_tensor(f"maskband{w}", [P, c.MB], BF16,
                             kind="ExternalInput") for w in range(2)]
    out_d = nc.dram_tensor("outT", [c.C, c.Tq], BF16, kind="ExternalOutput")

    with ExitStack() as ctx:
        tc = ctx.enter_context(tile.TileContext(nc))

        const_pool = ctx.enter_context(tc.tile_pool(name="const", bufs=1))
        ones_t = const_pool.tile([P, 1], F32)
        nc.vector.memset(ones_t[:], 1.0)
        zerob = const_pool.tile([P, 1], F32, name="zerob")
        nc.vector.memset(zerob[:], 0.0)
        epsb = const_pool.tile([1, 1], F32, name="epsb")
        nc.vector.memset(epsb[:], float(c.eps))
        ones_bf = const_pool.tile([P, 1], BF16, name="ones_bf")
        nc.vector.memset(ones_bf[:], 1.0)
        ones_r = const_pool.tile([P, 1], F32R, name="ones_r")
        nc.vector.tensor_copy(ones_r[:], ones_t[:])

        vec_tile = const_pool.tile([P, NV], F32, name="vecs")
        nc.sync.dma_start(out=vec_tile[:], in_=vecs_d.ap())
        _vo = [0]

        def vec_cols(n):
            k = n // P
            cols = [vec_tile[:, _vo[0] + i:_vo[0] + i + 1] for i in range(k)]
            _vo[0] += k
            return cols

        # LN affine gains/biases are folded into the weights host-side:
        # Wq/Wk/Wv carry ln1_g; qb = Wq^T ln1_b is added at the Q eviction;
        # the K bias cancels in softmax; the V bias flows linearly through
        # proj into bp; W1/b1 carry ln2_g/ln2_b.
        qb = vec_cols(c.C)
        bp, b1, b2 = vec_cols(c.C), vec_cols(c.F), vec_cols(c.C)

        # persistent state
        xq_pool = ctx.enter_context(tc.tile_pool(name="xq", bufs=1))
        xq_tiles = [xq_pool.tile([P, c.Tq], F32R, name=f"xq{i}")
                    for i in range(c.NC)]

        sap = ctx.enter_context(ExitStack())  # attn lifetime (thru proj)
        attn_pool = sap.enter_context(tc.tile_pool(name="attn", bufs=1))
        s1 = ctx.enter_context(ExitStack())  # q/k/v lifetime (thru attn)
        q_pool = s1.enter_context(tc.tile_pool(name="q", bufs=1))
        q_tiles = [q_pool.tile([P, c.Tq], BF16, name=f"q{i}")
                   for i in range(c.NC)]
        k_pool = s1.enter_context(tc.tile_pool(name="k", bufs=1))
        k_tiles = [k_pool.tile([P, c.Tkv], BF16, name=f"k{i}")
                   for i in range(c.NC)]
        v_pool = s1.enter_context(tc.tile_pool(name="v", bufs=1))
        v_tiles = [v_pool.tile([P, NH, 65], BF16, name=f"v{s}")
                   for s in range(c.NS)]

        # ---------- LN1 + QKV + attention, software-pipelined ----------
        # xqb (bf16) stages into the h1q/attn buffers so LN1q starts after a
        # 2MB DMA; the f32 residual xq streams in later (first used at proj).
        # Attention runs inside this scope so the second half of the V
        # projection can interleave into the attention-w0 loop, filling the
        # PE while the Activation engine (exp) is the bottleneck.
        TH = c.TH
        NS2 = c.NS // 2
        with ExitStack() as pkv:
            h1q_tiles = [attn_pool.tile([P, c.Tq], BF16, name=f"attn{i}")
                         for i in range(c.NC)]
            for ci in range(c.NC):
                nc.sync.dma_start(
                    out=h1q_tiles[ci][:],
                    in_=xqTb_d.ap()[ci * P:(ci + 1) * P, :])
            x_pool = pkv.enter_context(tc.tile_pool(name="xT", bufs=1))
            x_tiles = [x_pool.tile([P, c.Tkv], BF16, name=f"x{ci}")
                       for ci in range(c.NC)]
            for tt in range(4):
                for ci in range(c.NC):
                    nc.sync.dma_start(
                        out=x_tiles[ci][:, tt * 512:(tt + 1) * 512],
                        in_=xT_d.ap()[ci * P:(ci + 1) * P,
                                      tt * 512:(tt + 1) * 512])

            # long-lived pools first so they get fresh SBUF (a later creation
            # reuses the LN tmp pools' space and stalls its DMA behind them)
            w_pool = pkv.enter_context(tc.tile_pool(name="wqkv", bufs=1))
            mm_psum = pkv.enter_context(
                tc.tile_pool(name="kv_psum", bufs=1, space="PSUM"))
            w_tiles = []
            for ci in range(c.NC):
                wt = w_pool.tile([P, c.C], BF16, name=f"w{ci}", bufs=1)
                nc.sync.dma_start(
                    out=wt[:], in_=wq_d.ap()[ci * P:(ci + 1) * P, :])
                w_tiles.append(wt)
            mk_pool = pkv.enter_context(tc.tile_pool(name="mk", bufs=1))
            mask_t = [mk_pool.tile([P, c.MB], BF16, name=f"maskband{w}")
                      for w in range(2)]
            for w in range(2):
                nc.sync.dma_start(out=mask_t[w][:], in_=mask_d[w].ap())
            row_pool = pkv.enter_context(tc.tile_pool(name="lnrows", bufs=1))
            r1q = _ln_stats(nc, tc, c, h1q_tiles, c.Tq, ones_bf, zerob, epsb,
                            row_pool, "ln1q")
            r1 = _ln_stats(nc, tc, c, x_tiles, c.Tkv, ones_bf, zerob, epsb,
                           row_pool, "ln1")
            _ln_apply(nc, tc, c, h1q_tiles, h1q_tiles, c.Tq,
                      None, None, r1q, "ln1q")
            # LN1 apply in place (DVE) overlaps the Q matmuls (PE); Q psum
            # eviction goes through the Activation engine so DVE stays free
            _ln_apply(nc, tc, c, x_tiles, x_tiles, c.Tkv,
                      None, None, r1, "ln1")
            h1_tiles = x_tiles

            for fi in range(c.NC):
                pss = [mm_psum.tile([P, 512], F32,
                                    name=f"ps{(2 * fi + tt) % 4}")
                       for tt in range(2)]
                for ci in range(c.NC):
                    for tt in range(2):
                        nc.tensor.matmul(
                            pss[tt][:],
                            lhsT=w_tiles[ci][:, fi * P:(fi + 1) * P],
                            rhs=h1q_tiles[ci][:, tt * 512:(tt + 1) * 512],
                            start=(ci == 0), stop=(ci == c.NC - 1))
                for tt in range(2):
                    nc.scalar.activation(
                        q_tiles[fi][:, tt * 512:(tt + 1) * 512], pss[tt][:],
                        AF.Identity, bias=qb[fi][:])

            # residual xq (f32) arrives in the background; first use is proj
            for tt in range(2):
                for ci in range(c.NC):
                    nc.sync.dma_start(
                        out=xq_tiles[ci][:, tt * 512:(tt + 1) * 512],
                        in_=xqT_d.ap()[ci * P:(ci + 1) * P,
                                       tt * 512:(tt + 1) * 512])

            w_tiles = []
            for ci in range(c.NC):
                wt = w_pool.tile([P, c.C], BF16, name=f"w{ci}", bufs=1)
                nc.sync.dma_start(
                    out=wt[:], in_=wk_d.ap()[ci * P:(ci + 1) * P, :])
                w_tiles.append(wt)
            for fi in range(c.NC):
                pss = [mm_psum.tile([P, 512], F32,
                                    name=f"ps{(4 * fi + tt) % 6}")
                       for tt in range(4)]
                for ci in range(c.NC):
                    for tt in range(4):
                        nc.tensor.matmul(
                            pss[tt][:],
                            lhsT=w_tiles[ci][:, fi * P:(fi + 1) * P],
                            rhs=h1_tiles[ci][:, tt * 512:(tt + 1) * 512],
                            start=(ci == 0), stop=(ci == c.NC - 1))
                for tt in range(4):
                    nc.vector.tensor_copy(
                        k_tiles[fi][:, tt * 512:(tt + 1) * 512], pss[tt][:])

            wv_tiles = []
            for ci in range(c.NC):
                wt = w_pool.tile([P, c.C], BF16, name=f"w{ci}", bufs=1)
                nc.sync.dma_start(
                    out=wt[:], in_=wv_d.ap()[ci * P:(ci + 1) * P, :])
                wv_tiles.append(wt)
            FT = min(512, c.C)
            hpf = FT // 64
            NNF = c.C // FT

            def v_proj_part(s, nf, name):
                if nf == 0:
                    nc.vector.memset(v_tiles[s][:, :, 64:65], 1.0)
                psv = mm_psum.tile([P, FT], F32, name=name)
                for ci in range(c.NC):
                    nc.tensor.matmul(
                        psv[:],
                        lhsT=h1_tiles[ci][:, s * P:(s + 1) * P],
                        rhs=wv_tiles[ci][:, nf * FT:(nf + 1) * FT],
                        start=(ci == 0), stop=(ci == c.NC - 1))
                nc.vector.tensor_copy(
                    v_tiles[s][:, nf * hpf:(nf + 1) * hpf, 0:64],
                    psv[:].rearrange("p (h d) -> p h d", d=64))

            def v_proj(s, names):
                for nf in range(NNF):
                    v_proj_part(s, nf, names[nf])

            for s in range(NS2):
                v_proj(s, [f"ps{(NNF * s + nf) % 4}" for nf in range(NNF)])

            # ---------- attention (both halves) -> attn_sb ----------
            # e/r pools created late: no DMA in them, and the space they
            # reuse (LN tmps) is quiescent by the time attention runs
            e_pool = pkv.enter_context(tc.tile_pool(name="e", bufs=1))
            r_pool = pkv.enter_context(tc.tile_pool(name="r", bufs=1))
            attn_sb = [attn_pool.tile([P, c.Tq], BF16, name=f"attn{i}")
                       for i in range(c.NC)]

            def attn_head(w, hp, filler=None):
                wsl = slice(w * TH, (w + 1) * TH)
                avs = [mm_psum.tile([65, TH], F32, name=f"ps{4 + half}")
                       for half in range(2)]
                NJ = NS2 if w == 0 else c.NS
                # software-pipelined by one j-step: the AV matmul for step
                # j-1 issues after the scores for step j, so the PE never
                # waits on the exp/mask chain
                ets_prev = None
                for j in range(NJ):
                    ets = {}
                    for half in range(2):
                        hsl = slice(half * 64, half * 64 + 64)
                        ps = mm_psum.tile([P, TH], F32,
                                          name=f"ps{(2 * j + half) % 3}")
                        nc.tensor.matmul(
                            ps[:], lhsT=k_tiles[hp][hsl, j * P:(j + 1) * P],
                            rhs=q_tiles[hp][hsl, wsl],
                            start=True, stop=True)
                        et = e_pool.tile([P, TH], BF16,
                                         name=f"et{(2 * j + half) % 4}")
                        nc.scalar.activation(et[:], ps[:], AF.Exp,
                                             bias=zerob[:],
                                             scale=float(c.scale))
                        if w == 0 or j >= NS2:
                            cj = (c.Tkv - 128) - 128 * j - c.MLO[w]
                            nc.vector.tensor_tensor(
                                et[:], et[:],
                                mask_t[w][:, cj: cj + TH], op=ALU.mult)
                        ets[half] = (j, et)
                    if j > 0:
                        for half in range(2):
                            jj, et_p = ets_prev[half]
                            nc.tensor.matmul(
                                avs[half][:],
                                lhsT=v_tiles[jj][:, 2 * hp + half, :],
                                rhs=et_p[:], start=(jj == 0), stop=False)
                    if filler is not None and j in (2, 5):
                        filler(0 if j == 2 else 1)
                    ets_prev = dict(ets)
                for half in range(2):
                    jj, et_p = ets_prev[half]
                    nc.tensor.matmul(
                        avs[half][:],
                        lhsT=v_tiles[jj][:, 2 * hp + half, :],
                        rhs=et_p[:], start=(jj == 0), stop=True)
                for half in range(2):
                    av = avs[half]
                    hsl = slice(half * 64, half * 64 + 64)
                    rt0 = r_pool.tile([1, TH], F32, name="rt0")
                    nc.vector.reciprocal(rt0[:], av[64:65, :])
                    rb = r_pool.tile([64, TH], F32, name="rb")
                    nc.gpsimd.partition_broadcast(rb[:], rt0[:])
                    nc.vector.tensor_tensor(
                        attn_sb[hp][hsl, wsl], av[0:64, :], rb[:],
                        op=ALU.mult)

            # w=0 with the second half of the V projection interleaved (the
            # exp chain gates this half; V matmuls keep the PE busy)
            for hp in range(c.NC):
                attn_head(0, hp,
                          filler=lambda nf, s=NS2 + hp: v_proj_part(
                              s, nf, "ps3"))
            for hp in range(c.NC):
                attn_head(1, hp)

        s1.close()  # free q/k/v

        # ---------- proj + residual (full Tq, weights resident) ----------
        with ExitStack() as pd:
            pj_psum = pd.enter_context(
                tc.tile_pool(name="pj_psum", bufs=1, space="PSUM"))
            wp_pool = pd.enter_context(tc.tile_pool(name="wp", bufs=1))
            ev_pool = pd.enter_context(tc.tile_pool(name="pj_ev", bufs=2))
            wp_tiles = []
            for ci in range(c.NC):
                wt = wp_pool.tile([P, c.C], BF16, name=f"wp{ci}", bufs=1)
                nc.sync.dma_start(
                    out=wt[:], in_=wp_d.ap()[ci * P:(ci + 1) * P, :])
                wp_tiles.append(wt)
            for fi in range(c.NC):
                pss = [pj_psum.tile([P, 512], F32,
                                    name=f"pjp{(2 * fi + tt) % 6}")
                       for tt in range(2)]
                for ci in range(c.NC):
                    for tt in range(2):
                        nc.tensor.matmul(
                            pss[tt][:],
                            lhsT=wp_tiles[ci][:, fi * P:(fi + 1) * P],
                            rhs=attn_sb[ci][:, tt * 512:(tt + 1) * 512],
                            start=(ci == 0), stop=(ci == c.NC - 1))
                for tt in range(2):
                    tsl = slice(tt * 512, (tt + 1) * 512)
                    ev = ev_pool.tile([P, 512], F32, name="ev")
                    nc.vector.tensor_scalar(ev[:], pss[tt][:],
                                            bp[fi][:], None, op0=ALU.add)
                    nc.vector.tensor_tensor(
                        xq_tiles[fi][:, tsl], ev[:], xq_tiles[fi][:, tsl],
                        op=ALU.add)

        sap.close()  # free attn

        # ---------- LN2 (full Tq) ----------
        h2_pool = ctx.enter_context(tc.tile_pool(name="h2", bufs=1))
        h2_tiles = [h2_pool.tile([P, c.Tq], BF16, name=f"h2_{i}")
                    for i in range(c.NC)]
        x1_tiles = xq_tiles
        _layernorm_fm(nc, tc, c, x1_tiles, h2_tiles, c.Tq,
                      None, None, ones_r, zerob, epsb, "ln2")

        # ---------- FFN (full Tq, weights-outer, streamed) ----------
        relu_pool = ctx.enter_context(tc.tile_pool(name="relu", bufs=1))
        relu_tiles = [relu_pool.tile([P, c.Tq], BF16, name=f"r{i}")
                      for i in range(c.NF)]
        wst_pool = ctx.enter_context(tc.tile_pool(name="wst", bufs=1))
        ev_pool = ctx.enter_context(tc.tile_pool(name="ffn_ev", bufs=3))

        with ExitStack() as pw1:
            ff_psum = pw1.enter_context(
                tc.tile_pool(name="ff_psum", bufs=1, space="PSUM"))
            FG = min(512, c.F)
            for fg in range(c.F // FG):
                w1_tiles = []
                for c2 in range(c.NC // 2):
                    wt = wst_pool.tile([P, 2, FG], BF16,
                                       name=f"w1s{c2}", bufs=2)
                    nc.gpsimd.dma_start(
                        out=wt[:],
                        in_=w1_d.ap()[c2 * 2 * P:(c2 + 1) * 2 * P,
                                      fg * FG:(fg + 1) * FG]
                        .rearrange("(k p) f -> p k f", p=P))
                    w1_tiles.append(wt)
                for fi in range(FG // P):
                    f = fg * (FG // P) + fi
                    for tt in range(2):
                        psw = ff_psum.tile([P, 512], F32,
                                           name=f"psw{(2 * (fg * 4 + fi) + tt) % 6}")
                        for ci in range(c.NC):
                            nc.tensor.matmul(
                                psw[:],
                                lhsT=w1_tiles[ci // 2][:, ci % 2,
                                                       fi * P:(fi + 1) * P],
                                rhs=h2_tiles[ci][:, tt * 512:(tt + 1) * 512],
                                start=(ci == 0), stop=(ci == c.NC - 1))
                        nc.scalar.activation(
                            relu_tiles[f][:, tt * 512:(tt + 1) * 512],
                            psw[:], AF.Relu, bias=b1[f][:])

        # W2: four output column-groups of 2 C-tiles (4 psum banks each, pool
        # rotation overlaps group g's eviction with group g+1's matmuls);
        # w2 is still read exactly once (groups cover disjoint columns)
        with ExitStack() as pw2:
            w2_psum = pw2.enter_context(
                tc.tile_pool(name="w2_psum", bufs=2, space="PSUM"))
            for og in range(4):
                fis = range(og * 2, og * 2 + 2)
                pss = {(fi, tt): w2_psum.tile([P, 512], F32,
                                              name=f"ps2_{fi % 2}_{tt}")
                       for fi in fis for tt in range(2)}
                for c4 in range(c.NF // 4):
                    wt = wst_pool.tile([P, 4, 256], BF16, name="w2s", bufs=3)
                    nc.gpsimd.dma_start(
                        out=wt[:],
                        in_=w2_d.ap()[c4 * 4 * P:(c4 + 1) * 4 * P,
                                      og * 256:(og + 1) * 256]
                        .rearrange("(k p) f -> p k f", p=P))
                    for k in range(4):
                        ci = 4 * c4 + k
                        for fi in fis:
                            for tt in range(2):
                                nc.tensor.matmul(
                                    pss[(fi, tt)][:],
                                    lhsT=wt[:, k, (fi - og * 2) * P:
                                            (fi - og * 2 + 1) * P],
                                    rhs=relu_tiles[ci][:,
                                                       tt * 512:(tt + 1) * 512],
                                    start=(ci == 0),
                                    stop=(ci == c.NF - 1))
                for fi in fis:
                    for tt in range(2):
                        tsl = slice(tt * 512, (tt + 1) * 512)
                        ev = ev_pool.tile([P, 512], F32, name="ev2")
                        nc.vector.tensor_scalar(ev[:], pss[(fi, tt)][:],
                                                b2[fi][:], None, op0=ALU.add)
                        evb = ev_pool.tile([P, 512], BF16, name="evb")
                        nc.vector.tensor_tensor(
                            evb[:], ev[:], x1_tiles[fi][:, tsl], op=ALU.add)
                        nc.sync.dma_start(
                            out=out_d.ap()[fi * P:(fi + 1) * P, tsl],
                            in_=evb[:])
    return nc


def _ln_stats(nc, tc, c, x_tiles, T, ones_t, zerob, epsb, row_pool, name):
    """Per-token LN statistics -> (rs, -mu*rs) rows kept in row_pool.
    Rows are stored bf16 when the input is bf16 (the apply path broadcasts
    them in bf16 anyway)."""
    with ExitStack() as ctx:
        TT = min(512, T)
        NT = T // TT
        sq_pool = ctx.enter_context(tc.tile_pool(name=f"{name}_sq", bufs=1))
        st_psum = ctx.enter_context(
            tc.tile_pool(name=f"{name}_stp", bufs=1, space="PSUM"))
        tmp_pool = ctx.enter_context(tc.tile_pool(name=f"{name}_tmp", bufs=1))

        sq_dt = x_tiles[0].dtype
        row_dt = BF16 if sq_dt == BF16 else F32
        rs_row = row_pool.tile([1, T], row_dt, name=f"{name}_rs")
        nmrs_row = row_pool.tile([1, T], row_dt, name=f"{name}_nmrs")

        for tt in range(NT):
            sl = slice(tt * TT, (tt + 1) * TT)
            ps1 = st_psum.tile([1, TT], F32, name="ps1")
            ps2 = st_psum.tile([1, TT], F32, name="ps2")
            for ci, xt in enumerate(x_tiles):
                st, sp = ci == 0, ci == len(x_tiles) - 1
                nc.tensor.matmul(ps1[:], lhsT=ones_t[:],
                                 rhs=xt[:, sl], start=st, stop=sp)
                sq = sq_pool.tile([P, TT], sq_dt, name="sq")
                if sq_dt == BF16:
                    nc.vector.tensor_tensor(sq[:], xt[:, sl], xt[:, sl],
                                            op=ALU.mult)
                else:
                    nc.scalar.activation(sq[:], xt[:, sl], AF.Square,
                                         bias=zerob[:])
                nc.tensor.matmul(ps2[:], lhsT=ones_t[:], rhs=sq[:],
                                 start=st, stop=sp)
            mu = tmp_pool.tile([1, TT], F32, name="mu")
            nc.scalar.mul(mu[:], ps1[:], 1.0 / c.C)
            mu2 = tmp_pool.tile([1, TT], F32, name="mu2")
            nc.scalar.activation(mu2[:], mu[:], AF.Square, bias=zerob[0:1])
            var = tmp_pool.tile([1, TT], F32, name="var")
            nc.scalar.mul(var[:], ps2[:], 1.0 / c.C)
            nc.vector.tensor_sub(var[:], var[:], mu2[:])
            sd = tmp_pool.tile([1, TT], F32, name="sd")
            nc.scalar.activation(sd[:], var[:], AF.Sqrt, bias=epsb[:])
            if row_dt == F32:
                nc.vector.reciprocal(rs_row[:, sl], sd[:])
                nc.vector.tensor_tensor(nmrs_row[:, sl], mu[:],
                                        rs_row[:, sl], op=ALU.mult)
                nc.vector.tensor_scalar_mul(nmrs_row[:, sl],
                                            nmrs_row[:, sl], -1.0)
            else:
                # mu2 and var are dead: reuse as f32 scratch
                nc.vector.reciprocal(mu2[:], sd[:])
                nc.vector.tensor_copy(rs_row[:, sl], mu2[:])
                nc.vector.tensor_tensor(var[:], mu[:], mu2[:], op=ALU.mult)
                nc.vector.tensor_scalar_mul(var[:], var[:], -1.0)
                nc.vector.tensor_copy(nmrs_row[:, sl], var[:])
        return rs_row, nmrs_row


def _ln_apply(nc, tc, c, x_tiles, out_tiles, T, g_tiles, b_tiles, rows, name):
    rs_row, nmrs_row = rows
    with ExitStack() as ctx:
        TT = min(512, T)
        NT = T // TT
        tmp_pool = ctx.enter_context(tc.tile_pool(name=f"{name}_atmp", bufs=1))
        bc_dt = rs_row.dtype
        for tt in range(NT):
            sl = slice(tt * TT, (tt + 1) * TT)
            rs_b = tmp_pool.tile([P, TT], bc_dt, name="rsb", bufs=1)
            nmrs_b = tmp_pool.tile([P, TT], bc_dt, name="nmrsb", bufs=1)
            nc.gpsimd.partition_broadcast(rs_b[:], rs_row[:, sl])
            nc.gpsimd.partition_broadcast(nmrs_b[:], nmrs_row[:, sl])
            for ci, (xt, ot) in enumerate(zip(x_tiles, out_tiles)):
                nc.vector.tensor_tensor(ot[:, sl], xt[:, sl], rs_b[:],
                                        op=ALU.mult)
                nc.vector.tensor_tensor(ot[:, sl], ot[:, sl], nmrs_b[:],
                                        op=ALU.add)
                if g_tiles is not None:
                    nc.vector.tensor_scalar(ot[:, sl], ot[:, sl],
                                            g_tiles[ci][:], b_tiles[ci][:],
                                            op0=ALU.mult, op1=ALU.add)


def _layernorm_fm(nc, tc, c, x_tiles, out_tiles, T, g_tiles, b_tiles,
                  ones_t, zerob, epsb, name):
    with ExitStack() as ctx:
        row_pool = ctx.enter_context(
            tc.tile_pool(name=f"{name}_rows", bufs=1))
        rows = _ln_stats(nc, tc, c, x_tiles, T, ones_t, zerob, epsb,
                         row_pool, name)
        _ln_apply(nc, tc, c, x_tiles, out_tiles, T, g_tiles, b_tiles,
                  rows, name)


# ======================= host side =======================

def zigzag_tokens(cfg, p):
    c = cfg
    if p == 0:
        return np.concatenate([np.arange(0, c.TH),
                               np.arange(c.Tkv - c.TH, c.Tkv)])
    return np.arange(c.TH, c.TH + c.Tq)


def host_prepare(cfg, inputs, core_id):
    c = cfg
    b, p = core_id // 2, core_id % 2
    x = np.asarray(inputs["x"])
    xb = x[b]
    qidx = zigzag_tokens(c, p)
    xT = np.ascontiguousarray(xb.T).astype(ml_dtypes.bfloat16)
    xqT = np.ascontiguousarray(xb[qidx].T)

    def flat_w(w):
        return np.ascontiguousarray(
            np.transpose(np.asarray(w), (1, 0, 2)).reshape(c.C, c.C))

    bf = lambda a: np.ascontiguousarray(
        np.asarray(a).astype(ml_dtypes.bfloat16))
    g = np.arange(c.MB)[None, :]
    pp = np.arange(128)[:, None]
    qoff_w = ((0, c.Tkv - c.TH), (c.TH, c.Tq))[p]
    bands = [(g + c.MLO[w] >= pp + (c.Tkv - 128) - qoff_w[w])
             .astype(np.float32) for w in range(2)]

    # fold LN affines into the weights (see build_kernel docstring)
    g1 = np.asarray(inputs["ln1_g"], np.float64)
    b1v = np.asarray(inputs["ln1_b"], np.float64)
    g2 = np.asarray(inputs["ln2_g"], np.float64)
    b2v = np.asarray(inputs["ln2_b"], np.float64)
    fq = flat_w(inputs["Wq"]).astype(np.float64)
    fk = flat_w(inputs["Wk"]).astype(np.float64)
    fv = flat_w(inputs["Wv"]).astype(np.float64)
    wp = np.asarray(inputs["Wp"], np.float64)
    w1 = np.asarray(inputs["W1"], np.float64)
    qb = b1v @ fq
    vb = b1v @ fv
    bp_f = np.asarray(inputs["bp"], np.float64) + vb @ wp
    b1_f = np.asarray(inputs["b1"], np.float64) + b2v @ w1

    vl = [qb.astype(np.float32).reshape(-1, 128).T,
          bp_f.astype(np.float32).reshape(-1, 128).T,
          b1_f.astype(np.float32).reshape(-1, 128).T,
          np.asarray(inputs["b2"], np.float32).reshape(-1, 128).T]
    vecs = np.ascontiguousarray(np.concatenate(vl, axis=1))
    return {
        "xT": xT, "xqT": xqT,
        "xqTb": np.ascontiguousarray(xb[qidx].T).astype(ml_dtypes.bfloat16),
        "wq": bf(fq * g1[:, None]), "wk": bf(fk * g1[:, None]),
        "wv": bf(fv * g1[:, None]),
        "wp": bf(wp), "w1": bf(w1 * g2[:, None]),
        "w2": bf(inputs["W2"]),
        "vecs": vecs,
        "maskband0": bf(bands[0]),
        "maskband1": bf(bands[1]),
    }


def host_gather(cfg, results, B):
    c = cfg
    out = np.empty((B, c.Tkv, c.C), np.float32)
    for core in range(2 * B):
        b, p = core // 2, core % 2
        out[b, zigzag_tokens(c, p), :] = \
            np.asarray(results[core]["outT"]).astype(np.float32).T
    return out


_CACHE = {}


def _get_compiled(n_cores=8, ln_affine=True):
    key = "nc"
    if key not in _CACHE:
        cfg = Cfg(C=1024, H=16, D=64, Tkv=2048)
        nc = bacc.Bacc("TRN2", target_bir_lowering=False, debug=False,
                       num_devices=n_cores)
        build_kernel(nc, cfg)
        nc.compile()
        _CACHE[key] = (nc, cfg)
    return _CACHE[key]


def kernel(**inputs):
    """Full transformer block on 8 NeuronCores. Takes the full unsharded
    inputs (as in reference.setup_inputs) and returns the full [4, 2048,
    1024] float32 output."""
    nc, cfg = _get_compiled(8)
    x = np.asarray(inputs["x"])
    B = x.shape[0]
    n_cores = 2 * B
    in_maps = [host_prepare(cfg, inputs, core) for core in range(n_cores)]
    res = run_bass_kernel_spmd(nc, in_maps, core_ids=list(range(n_cores)))
    return host_gather(cfg, res.results, B)


# revision 38
# speedup vs baseline: 1.2967x; 1.2967x over previous
"""TRN2 Bass/Tile kernel for nn_Block_89842125898023 (dense transformer
block), SPMD over 8 NeuronCores.

Sharding (data-parallel over batch x query-halves, zero collectives):
core c handles batch element b = c//2 and query half p = c%2 of that
element's 2048 tokens, using a "zigzag" split (p=0: tokens [0,512) u
[1536,2048); p=1: [512,1536)) so the causal-attention work is identical
on every core. Each core redundantly computes K/V for its batch
element's full sequence from the (replicated) xT input — cheaper than
any cross-core collective.

On-device layout is feature-major ([C, T], channels on partitions).
Q/K/attn stay SBUF-resident (no DRAM round-trips); attention computes
transposed scores S^T[s, t] per head, the softmax normalizer Z comes
from a ones-column appended to V (M=65 AV matmul), the causal mask is
applied multiplicatively post-exp from per-core band-mask inputs, and
the division by Z is deferred to the AV eviction. All matmuls run in
bf16 with fp32 PSUM accumulation; proj and the FFN iterate weights-
outer over the full 1024-token query block so every weight byte is
read from DRAM exactly once. LayerNorm statistics are computed with
ones-vector matmuls on the PE; 1/sqrt(var+eps) is computed as
exp(-0.5*ln(var+eps)) so the whole kernel uses a single activation
table (no LoadActFuncSet churn).

kernel(**inputs) takes the full unsharded inputs, builds per-core input
maps host-side, runs the SPMD program on cores 0-7 via
bass_utils.run_bass_kernel_spmd, and reassembles the full output.
"""

import sys
import os

sys.path.insert(0, "/opt/trn_rl_repo")

from contextlib import ExitStack

import numpy as np
import ml_dtypes

import concourse.bass as bass
import concourse.bacc as bacc
import concourse.tile as tile
from concourse import mybir
from concourse.bass_utils import run_bass_kernel_spmd

F32 = mybir.dt.float32
F32R = mybir.dt.float32r
BF16 = mybir.dt.bfloat16
AF = mybir.ActivationFunctionType
ALU = mybir.AluOpType
P = 128


class Cfg:
    def __init__(self, C=1024, H=16, D=64, Tkv=2048, eps=1e-5, ffn_mult=4):
        self.C = C
        self.H = H
        self.D = D
        assert H * D == C
        self.Tkv = Tkv
        self.Tq = Tkv // 2
        self.F = ffn_mult * C
        self.eps = eps
        self.NC = C // 128
        self.NF = self.F // 128
        self.NS = Tkv // 128
        self.scale = C ** -0.5
        self.TH = self.Tq // 2
        NS2 = self.NS // 2
        self.MB = 128 * (NS2 - 1) + self.TH
        self.MLO = ((Tkv - 128) - 128 * (NS2 - 1), 0)


def build_kernel(nc: bass.Bass, cfg: Cfg, ln_affine=True):
    c = cfg
    NH = c.C // 64

    xT_d = nc.dram_tensor("xT", [c.C, c.Tkv], BF16, kind="ExternalInput")
    xqT_d = nc.dram_tensor("xqT", [c.C, c.Tq], F32R, kind="ExternalInput")
    xqTb_d = nc.dram_tensor("xqTb", [c.C, c.Tq], BF16, kind="ExternalInput")
    wq_d = nc.dram_tensor("wq", [c.C, c.C], BF16, kind="ExternalInput")
    wk_d = nc.dram_tensor("wk", [c.C, c.C], BF16, kind="ExternalInput")
    wv_d = nc.dram_tensor("wv", [c.C, c.C], BF16, kind="ExternalInput")
    wp_d = nc.dram_tensor("wp", [c.C, c.C], BF16, kind="ExternalInput")
    w1_d = nc.dram_tensor("w1", [c.C, c.F], BF16, kind="ExternalInput")
    w2_d = nc.dram_tensor("w2", [c.F, c.C], BF16, kind="ExternalInput")
    NV = 3 * (c.C // P) + c.F // P
    vecs_d = nc.dram_tensor("vecs", [P, NV], F32, kind="ExternalInput")
    mask_d = [nc.dram_tensor(f"maskband{w}", [P, c.MB], BF16,
                             kind="ExternalInput") for w in range(2)]
    out_d = nc.dram_tensor("outT", [c.C, c.Tq], BF16, kind="ExternalOutput")

    with ExitStack() as ctx:
        tc = ctx.enter_context(tile.TileContext(nc))

        const_pool = ctx.enter_context(tc.tile_pool(name="const", bufs=1))
        ones_t = const_pool.tile([P, 1], F32)
        nc.vector.memset(ones_t[:], 1.0)
        zerob = const_pool.tile([P, 1], F32, name="zerob")
        nc.vector.memset(zerob[:], 0.0)
        epsb = const_pool.tile([1, 1], F32, name="epsb")
        nc.vector.memset(epsb[:], float(c.eps))
        ones_bf = const_pool.tile([P, 1], BF16, name="ones_bf")
        nc.vector.memset(ones_bf[:], 1.0)
        ones_r = const_pool.tile([P, 1], F32R, name="ones_r")
        nc.vector.tensor_copy(ones_r[:], ones_t[:])

        vec_tile = const_pool.tile([P, NV], F32, name="vecs")
        nc.sync.dma_start(out=vec_tile[:], in_=vecs_d.ap())
        _vo = [0]

        def vec_cols(n):
            k = n // P
            cols = [vec_tile[:, _vo[0] + i:_vo[0] + i + 1] for i in range(k)]
            _vo[0] += k
            return cols

        # LN affine gains/biases are folded into the weights host-side:
        # Wq/Wk/Wv carry ln1_g; qb = Wq^T ln1_b is added at the Q eviction;
        # the K bias cancels in softmax; the V bias flows linearly through
        # proj into bp; W1/b1 carry ln2_g/ln2_b.
        qb = vec_cols(c.C)
        bp, b1, b2 = vec_cols(c.C), vec_cols(c.F), vec_cols(c.C)

        # persistent state
        xq_pool = ctx.enter_context(tc.tile_pool(name="xq", bufs=1))
        xq_tiles = [xq_pool.tile([P, c.Tq], F32R, name=f"xq{i}")
                    for i in range(c.NC)]

        sap = ctx.enter_context(ExitStack())  # attn lifetime (thru proj)
        attn_pool = sap.enter_context(tc.tile_pool(name="attn", bufs=1))
        s1 = ctx.enter_context(ExitStack())  # q/k/v lifetime (thru attn)
        q_pool = s1.enter_context(tc.tile_pool(name="q", bufs=1))
        q_tiles = [q_pool.tile([P, c.Tq], BF16, name=f"q{i}")
                   for i in range(c.NC)]
        k_pool = s1.enter_context(tc.tile_pool(name="k", bufs=1))
        k_tiles = [k_pool.tile([P, c.Tkv], BF16, name=f"k{i}")
                   for i in range(c.NC)]
        v_pool = s1.enter_context(tc.tile_pool(name="v", bufs=1))
        v_tiles = [v_pool.tile([P, NH, 65], BF16, name=f"v{s}")
                   for s in range(c.NS)]

        # ---------- LN1 + QKV + attention, software-pipelined ----------
        # xqb (bf16) stages into the h1q/attn buffers so LN1q starts after a
        # 2MB DMA; the f32 residual xq streams in later (first used at proj).
        # Attention runs inside this scope so the second half of the V
        # projection can interleave into the attention-w0 loop, filling the
        # PE while the Activation engine (exp) is the bottleneck.
        TH = c.TH
        NS2 = c.NS // 2
        with ExitStack() as pkv:
            h1q_tiles = [attn_pool.tile([P, c.Tq], BF16, name=f"attn{i}")
                         for i in range(c.NC)]
            for ci in range(c.NC):
                nc.sync.dma_start(
                    out=h1q_tiles[ci][:],
                    in_=xqTb_d.ap()[ci * P:(ci + 1) * P, :])
            x_pool = pkv.enter_context(tc.tile_pool(name="xT", bufs=1))
            x_tiles = [x_pool.tile([P, c.Tkv], BF16, name=f"x{ci}")
                       for ci in range(c.NC)]
            for tt in range(4):
                for ci in range(c.NC):
                    nc.sync.dma_start(
                        out=x_tiles[ci][:, tt * 512:(tt + 1) * 512],
                        in_=xT_d.ap()[ci * P:(ci + 1) * P,
                                      tt * 512:(tt + 1) * 512])

            # long-lived pools first so they get fresh SBUF (a later creation
            # reuses the LN tmp pools' space and stalls its DMA behind them)
            w_pool = pkv.enter_context(tc.tile_pool(name="wqkv", bufs=1))
            mm_psum = pkv.enter_context(
                tc.tile_pool(name="kv_psum", bufs=1, space="PSUM"))
            w_tiles = []
            for ci in range(c.NC):
                wt = w_pool.tile([P, c.C], BF16, name=f"w{ci}", bufs=1)
                nc.sync.dma_start(
                    out=wt[:], in_=wq_d.ap()[ci * P:(ci + 1) * P, :])
                w_tiles.append(wt)
            mk_pool = pkv.enter_context(tc.tile_pool(name="mk", bufs=1))
            mask_t = [mk_pool.tile([P, c.MB], BF16, name=f"maskband{w}")
                      for w in range(2)]
            for w in range(2):
                nc.sync.dma_start(out=mask_t[w][:], in_=mask_d[w].ap())
            row_pool = pkv.enter_context(tc.tile_pool(name="lnrows", bufs=1))
            r1q = _ln_stats(nc, tc, c, h1q_tiles, c.Tq, ones_bf, zerob, epsb,
                            row_pool, "ln1q")
            r1 = _ln_stats(nc, tc, c, x_tiles, c.Tkv, ones_bf, zerob, epsb,
                           row_pool, "ln1")
            _ln_apply(nc, tc, c, h1q_tiles, h1q_tiles, c.Tq,
                      None, None, r1q, "ln1q")
            # LN1 apply in place (DVE) overlaps the Q matmuls (PE); Q psum
            # eviction goes through the Activation engine so DVE stays free
            _ln_apply(nc, tc, c, x_tiles, x_tiles, c.Tkv,
                      None, None, r1, "ln1")
            h1_tiles = x_tiles

            for fi in range(c.NC):
                pss = [mm_psum.tile([P, 512], F32,
                                    name=f"ps{(2 * fi + tt) % 4}")
                       for tt in range(2)]
                for ci in range(c.NC):
                    for tt in range(2):
                        nc.tensor.matmul(
                            pss[tt][:],
                            lhsT=w_tiles[ci][:, fi * P:(fi + 1) * P],
                            rhs=h1q_tiles[ci][:, tt * 512:(tt + 1) * 512],
                            start=(ci == 0), stop=(ci == c.NC - 1))
                for tt in range(2):
                    nc.scalar.activation(
                        q_tiles[fi][:, tt * 512:(tt + 1) * 512], pss[tt][:],
                        AF.Identity, bias=qb[fi][:])

            # residual xq (f32) arrives in the background; first use is proj
            for tt in range(2):
                for ci in range(c.NC):
                    nc.sync.dma_start(
                        out=xq_tiles[ci][:, tt * 512:(tt + 1) * 512],
                        in_=xqT_d.ap()[ci * P:(ci + 1) * P,
                                       tt * 512:(tt + 1) * 512])

            w_tiles = []
            for ci in range(c.NC):
                wt = w_pool.tile([P, c.C], BF16, name=f"w{ci}", bufs=1)
                nc.sync.dma_start(
                    out=wt[:], in_=wk_d.ap()[ci * P:(ci + 1) * P, :])
                w_tiles.append(wt)
            for fi in range(c.NC):
                pss = [mm_psum.tile([P, 512], F32,
                                    name=f"ps{(4 * fi + tt) % 6}")
                       for tt in range(4)]
                for ci in range(c.NC):
                    for tt in range(4):
                        nc.tensor.matmul(
                            pss[tt][:],
                            lhsT=w_tiles[ci][:, fi * P:(fi + 1) * P],
                            rhs=h1_tiles[ci][:, tt * 512:(tt + 1) * 512],
                            start=(ci == 0), stop=(ci == c.NC - 1))
                for tt in range(4):
                    nc.vector.tensor_copy(
                        k_tiles[fi][:, tt * 512:(tt + 1) * 512], pss[tt][:])

            wv_tiles = []
            for ci in range(c.NC):
                wt = w_pool.tile([P, c.C], BF16, name=f"w{ci}", bufs=1)
                nc.sync.dma_start(
                    out=wt[:], in_=wv_d.ap()[ci * P:(ci + 1) * P, :])
                wv_tiles.append(wt)
            FT = min(512, c.C)
            hpf = FT // 64
            NNF = c.C // FT

            def v_proj_part(s, nf, name):
                if nf == 0:
                    nc.vector.memset(v_tiles[s][:, :, 64:65], 1.0)
                psv = mm_psum.tile([P, FT], F32, name=name)
                for ci in range(c.NC):
                    nc.tensor.matmul(
                        psv[:],
                        lhsT=h1_tiles[ci][:, s * P:(s + 1) * P],
                        rhs=wv_tiles[ci][:, nf * FT:(nf + 1) * FT],
                        start=(ci == 0), stop=(ci == c.NC - 1))
                nc.vector.tensor_copy(
                    v_tiles[s][:, nf * hpf:(nf + 1) * hpf, 0:64],
                    psv[:].rearrange("p (h d) -> p h d", d=64))

            def v_proj(s, names):
                for nf in range(NNF):
                    v_proj_part(s, nf, names[nf])

            for s in range(NS2):
                v_proj(s, [f"ps{(NNF * s + nf) % 4}" for nf in range(NNF)])

            # ---------- attention (both halves) -> attn_sb ----------
            # e/r pools created late: no DMA in them, and the space they
            # reuse (LN tmps) is quiescent by the time attention runs
            e_pool = pkv.enter_context(tc.tile_pool(name="e", bufs=1))
            r_pool = pkv.enter_context(tc.tile_pool(name="r", bufs=1))
            attn_sb = [attn_pool.tile([P, c.Tq], BF16, name=f"attn{i}")
                       for i in range(c.NC)]

            def attn_head(w, hp, filler=None):
                wsl = slice(w * TH, (w + 1) * TH)
                avs = [mm_psum.tile([65, TH], F32, name=f"ps{4 + half}")
                       for half in range(2)]
                NJ = NS2 if w == 0 else c.NS
                # software-pipelined by one j-step: the AV matmul for step
                # j-1 issues after the scores for step j, so the PE never
                # waits on the exp/mask chain
                ets_prev = None
                for j in range(NJ):
                    ets = {}
                    for half in range(2):
                        hsl = slice(half * 64, half * 64 + 64)
                        ps = mm_psum.tile([P, TH], F32,
                                          name=f"ps{(2 * j + half) % 3}")
                        nc.tensor.matmul(
                            ps[:], lhsT=k_tiles[hp][hsl, j * P:(j + 1) * P],
                            rhs=q_tiles[hp][hsl, wsl],
                            start=True, stop=True)
                        et = e_pool.tile([P, TH], BF16,
                                         name=f"et{(2 * j + half) % 4}")
                        nc.scalar.activation(et[:], ps[:], AF.Exp,
                                             bias=zerob[:],
                                             scale=float(c.scale))
                        if w == 0 or j >= NS2:
                            cj = (c.Tkv - 128) - 128 * j - c.MLO[w]
                            nc.vector.tensor_tensor(
                                et[:], et[:],
                                mask_t[w][:, cj: cj + TH], op=ALU.mult)
                        ets[half] = (j, et)
                    if j > 0:
                        for half in range(2):
                            jj, et_p = ets_prev[half]
                            nc.tensor.matmul(
                                avs[half][:],
                                lhsT=v_tiles[jj][:, 2 * hp + half, :],
                                rhs=et_p[:], start=(jj == 0), stop=False)
                    if filler is not None and j in (2, 5):
                        filler(0 if j == 2 else 1)
                    ets_prev = dict(ets)
                for half in range(2):
                    jj, et_p = ets_prev[half]
                    nc.tensor.matmul(
                        avs[half][:],
                        lhsT=v_tiles[jj][:, 2 * hp + half, :],
                        rhs=et_p[:], start=(jj == 0), stop=True)
                for half in range(2):
                    av = avs[half]
                    hsl = slice(half * 64, half * 64 + 64)
                    rt0 = r_pool.tile([1, TH], F32, name="rt0")
                    nc.vector.reciprocal(rt0[:], av[64:65, :])
                    rb = r_pool.tile([64, TH], F32, name="rb")
                    nc.gpsimd.partition_broadcast(rb[:], rt0[:])
                    nc.vector.tensor_tensor(
                        attn_sb[hp][hsl, wsl], av[0:64, :], rb[:],
                        op=ALU.mult)

            # w=0 with the second half of the V projection interleaved (the
            # exp chain gates this half; V matmuls keep the PE busy)
            for hp in range(c.NC):
                attn_head(0, hp,
                          filler=lambda nf, s=NS2 + hp: v_proj_part(
                              s, nf, "ps3"))
            for hp in range(c.NC):
                attn_head(1, hp)

        s1.close()  # free q/k/v

        # ---------- proj + residual (full Tq, weights resident) ----------
        with ExitStack() as pd:
            pj_psum = pd.enter_context(
                tc.tile_pool(name="pj_psum", bufs=1, space="PSUM"))
            wp_pool = pd.enter_context(tc.tile_pool(name="wp", bufs=1))
            ev_pool = pd.enter_context(tc.tile_pool(name="pj_ev", bufs=2))
            wp_tiles = []
            for ci in range(c.NC):
                wt = wp_pool.tile([P, c.C], BF16, name=f"wp{ci}", bufs=1)
                nc.sync.dma_start(
                    out=wt[:], in_=wp_d.ap()[ci * P:(ci + 1) * P, :])
                wp_tiles.append(wt)
            for fi in range(c.NC):
                pss = [pj_psum.tile([P, 512], F32,
                                    name=f"pjp{(2 * fi + tt) % 6}")
                       for tt in range(2)]
                for ci in range(c.NC):
                    for tt in range(2):
                        nc.tensor.matmul(
                            pss[tt][:],
                            lhsT=wp_tiles[ci][:, fi * P:(fi + 1) * P],
                            rhs=attn_sb[ci][:, tt * 512:(tt + 1) * 512],
                            start=(ci == 0), stop=(ci == c.NC - 1))
                for tt in range(2):
                    tsl = slice(tt * 512, (tt + 1) * 512)
                    nc.vector.scalar_tensor_tensor(
                        xq_tiles[fi][:, tsl], pss[tt][:], bp[fi][:],
                        xq_tiles[fi][:, tsl], op0=ALU.add, op1=ALU.add)

        sap.close()  # free attn

        # ---------- LN2 (full Tq) ----------
        h2_pool = ctx.enter_context(tc.tile_pool(name="h2", bufs=1))
        h2_tiles = [h2_pool.tile([P, c.Tq], BF16, name=f"h2_{i}")
                    for i in range(c.NC)]
        x1_tiles = xq_tiles
        _layernorm_fm(nc, tc, c, x1_tiles, h2_tiles, c.Tq,
                      None, None, ones_r, zerob, epsb, "ln2")

        # ---------- FFN (full Tq, weights-outer, streamed) ----------
        relu_pool = ctx.enter_context(tc.tile_pool(name="relu", bufs=1))
        relu_tiles = [relu_pool.tile([P, c.Tq], BF16, name=f"r{i}")
                      for i in range(c.NF)]
        wst_pool = ctx.enter_context(tc.tile_pool(name="wst", bufs=1))
        ev_pool = ctx.enter_context(tc.tile_pool(name="ffn_ev", bufs=3))

        with ExitStack() as pw1:
            ff_psum = pw1.enter_context(
                tc.tile_pool(name="ff_psum", bufs=1, space="PSUM"))
            FG = min(512, c.F)
            for fg in range(c.F // FG):
                w1_tiles = []
                for c2 in range(c.NC // 2):
                    wt = wst_pool.tile([P, 2, FG], BF16,
                                       name=f"w1s{c2}", bufs=2)
                    nc.gpsimd.dma_start(
                        out=wt[:],
                        in_=w1_d.ap()[c2 * 2 * P:(c2 + 1) * 2 * P,
                                      fg * FG:(fg + 1) * FG]
                        .rearrange("(k p) f -> p k f", p=P))
                    w1_tiles.append(wt)
                for fi in range(FG // P):
                    f = fg * (FG // P) + fi
                    for tt in range(2):
                        psw = ff_psum.tile([P, 512], F32,
                                           name=f"psw{(2 * (fg * 4 + fi) + tt) % 6}")
                        for ci in range(c.NC):
                            nc.tensor.matmul(
                                psw[:],
                                lhsT=w1_tiles[ci // 2][:, ci % 2,
                                                       fi * P:(fi + 1) * P],
                                rhs=h2_tiles[ci][:, tt * 512:(tt + 1) * 512],
                                start=(ci == 0), stop=(ci == c.NC - 1))
                        nc.scalar.activation(
                            relu_tiles[f][:, tt * 512:(tt + 1) * 512],
                            psw[:], AF.Relu, bias=b1[f][:])

        # W2: four output column-groups of 2 C-tiles (4 psum banks each, pool
        # rotation overlaps group g's eviction with group g+1's matmuls);
        # w2 is still read exactly once (groups cover disjoint columns)
        with ExitStack() as pw2:
            w2_psum = pw2.enter_context(
                tc.tile_pool(name="w2_psum", bufs=2, space="PSUM"))
            for og in range(4):
                fis = range(og * 2, og * 2 + 2)
                pss = {(fi, tt): w2_psum.tile([P, 512], F32,
                                              name=f"ps2_{fi % 2}_{tt}")
                       for fi in fis for tt in range(2)}
                for c4 in range(c.NF // 4):
                    wt = wst_pool.tile([P, 4, 256], BF16, name="w2s", bufs=3)
                    nc.gpsimd.dma_start(
                        out=wt[:],
                        in_=w2_d.ap()[c4 * 4 * P:(c4 + 1) * 4 * P,
                                      og * 256:(og + 1) * 256]
                        .rearrange("(k p) f -> p k f", p=P))
                    for k in range(4):
                        ci = 4 * c4 + k
                        for fi in fis:
                            for tt in range(2):
                                nc.tensor.matmul(
                                    pss[(fi, tt)][:],
                                    lhsT=wt[:, k, (fi - og * 2) * P:
                                            (fi - og * 2 + 1) * P],
                                    rhs=relu_tiles[ci][:,
                                                       tt * 512:(tt + 1) * 512],
                                    start=(ci == 0),
                                    stop=(ci == c.NF - 1))
                for fi in fis:
                    for tt in range(2):
                        tsl = slice(tt * 512, (tt + 1) * 512)
                        evb = ev_pool.tile([P, 512], BF16, name="evb")
                        nc.vector.scalar_tensor_tensor(
                            evb[:], pss[(fi, tt)][:], b2[fi][:],
                            x1_tiles[fi][:, tsl], op0=ALU.add, op1=ALU.add)
                        nc.sync.dma_start(
                            out=out_d.ap()[fi * P:(fi + 1) * P, tsl],
                            in_=evb[:])
    return nc


def _ln_stats(nc, tc, c, x_tiles, T, ones_t, zerob, epsb, row_pool, name):
    """Per-token LN statistics -> (rs, -mu*rs) rows kept in row_pool.
    Rows are stored bf16 when the input is bf16 (the apply path broadcasts
    them in bf16 anyway)."""
    with ExitStack() as ctx:
        TT = min(512, T)
        NT = T // TT
        sq_pool = ctx.enter_context(tc.tile_pool(name=f"{name}_sq", bufs=1))
        st_psum = ctx.enter_context(
            tc.tile_pool(name=f"{name}_stp", bufs=1, space="PSUM"))
        tmp_pool = ctx.enter_context(tc.tile_pool(name=f"{name}_tmp", bufs=1))

        sq_dt = x_tiles[0].dtype
        row_dt = BF16 if sq_dt == BF16 else F32
        rs_row = row_pool.tile([1, T], row_dt, name=f"{name}_rs")
        nmrs_row = row_pool.tile([1, T], row_dt, name=f"{name}_nmrs")

        for tt in range(NT):
            sl = slice(tt * TT, (tt + 1) * TT)
            ps1 = st_psum.tile([1, TT], F32, name="ps1")
            ps2 = st_psum.tile([1, TT], F32, name="ps2")
            for ci, xt in enumerate(x_tiles):
                st, sp = ci == 0, ci == len(x_tiles) - 1
                nc.tensor.matmul(ps1[:], lhsT=ones_t[:],
                                 rhs=xt[:, sl], start=st, stop=sp)
                sq = sq_pool.tile([P, TT], sq_dt, name="sq")
                if sq_dt == BF16:
                    nc.vector.tensor_tensor(sq[:], xt[:, sl], xt[:, sl],
                                            op=ALU.mult)
                else:
                    nc.scalar.activation(sq[:], xt[:, sl], AF.Square,
                                         bias=zerob[:])
                nc.tensor.matmul(ps2[:], lhsT=ones_t[:], rhs=sq[:],
                                 start=st, stop=sp)
            mu = tmp_pool.tile([1, TT], F32, name="mu")
            nc.scalar.mul(mu[:], ps1[:], 1.0 / c.C)
            mu2 = tmp_pool.tile([1, TT], F32, name="mu2")
            nc.scalar.activation(mu2[:], mu[:], AF.Square, bias=zerob[0:1])
            var = tmp_pool.tile([1, TT], F32, name="var")
            nc.scalar.mul(var[:], ps2[:], 1.0 / c.C)
            nc.vector.tensor_sub(var[:], var[:], mu2[:])
            sd = tmp_pool.tile([1, TT], F32, name="sd")
            nc.scalar.activation(sd[:], var[:], AF.Sqrt, bias=epsb[:])
            if row_dt == F32:
                nc.vector.reciprocal(rs_row[:, sl], sd[:])
                nc.vector.tensor_tensor(nmrs_row[:, sl], mu[:],
                                        rs_row[:, sl], op=ALU.mult)
                nc.vector.tensor_scalar_mul(nmrs_row[:, sl],
                                            nmrs_row[:, sl], -1.0)
            else:
                # mu2 and var are dead: reuse as f32 scratch
                nc.vector.reciprocal(mu2[:], sd[:])
                nc.vector.tensor_copy(rs_row[:, sl], mu2[:])
                nc.vector.tensor_tensor(var[:], mu[:], mu2[:], op=ALU.mult)
                nc.vector.tensor_scalar_mul(var[:], var[:], -1.0)
                nc.vector.tensor_copy(nmrs_row[:, sl], var[:])
        return rs_row, nmrs_row


def _ln_apply(nc, tc, c, x_tiles, out_tiles, T, g_tiles, b_tiles, rows, name):
    rs_row, nmrs_row = rows
    with ExitStack() as ctx:
        TT = min(512, T)
        NT = T // TT
        tmp_pool = ctx.enter_context(tc.tile_pool(name=f"{name}_atmp", bufs=1))
        bc_dt = rs_row.dtype
        for tt in range(NT):
            sl = slice(tt * TT, (tt + 1) * TT)
            rs_b = tmp_pool.tile([P, TT], bc_dt, name="rsb", bufs=1)
            nmrs_b = tmp_pool.tile([P, TT], bc_dt, name="nmrsb", bufs=1)
            nc.gpsimd.partition_broadcast(rs_b[:], rs_row[:, sl])
            nc.gpsimd.partition_broadcast(nmrs_b[:], nmrs_row[:, sl])
            for ci, (xt, ot) in enumerate(zip(x_tiles, out_tiles)):
                nc.vector.tensor_tensor(ot[:, sl], xt[:, sl], rs_b[:],
                                        op=ALU.mult)
                nc.vector.tensor_tensor(ot[:, sl], ot[:, sl], nmrs_b[:],
                                        op=ALU.add)
                if g_tiles is not None:
                    nc.vector.tensor_scalar(ot[:, sl], ot[:, sl],
                                            g_tiles[ci][:], b_tiles[ci][:],
                                            op0=ALU.mult, op1=ALU.add)


def _layernorm_fm(nc, tc, c, x_tiles, out_tiles, T, g_tiles, b_tiles,
                  ones_t, zerob, epsb, name):
    with ExitStack() as ctx:
        row_pool = ctx.enter_context(
            tc.tile_pool(name=f"{name}_rows", bufs=1))
        rows = _ln_stats(nc, tc, c, x_tiles, T, ones_t, zerob, epsb,
                         row_pool, name)
        _ln_apply(nc, tc, c, x_tiles, out_tiles, T, g_tiles, b_tiles,
                  rows, name)


# ======================= host side =======================

def zigzag_tokens(cfg, p):
    c = cfg
    if p == 0:
        return np.concatenate([np.arange(0, c.TH),
                               np.arange(c.Tkv - c.TH, c.Tkv)])
    return np.arange(c.TH, c.TH + c.Tq)


def host_prepare(cfg, inputs, core_id):
    c = cfg
    b, p = core_id // 2, core_id % 2
    x = np.asarray(inputs["x"])
    xb = x[b]
    qidx = zigzag_tokens(c, p)
    xT = np.ascontiguousarray(xb.T).astype(ml_dtypes.bfloat16)
    xqT = np.ascontiguousarray(xb[qidx].T)

    def flat_w(w):
        return np.ascontiguousarray(
            np.transpose(np.asarray(w), (1, 0, 2)).reshape(c.C, c.C))

    bf = lambda a: np.ascontiguousarray(
        np.asarray(a).astype(ml_dtypes.bfloat16))
    g = np.arange(c.MB)[None, :]
    pp = np.arange(128)[:, None]
    qoff_w = ((0, c.Tkv - c.TH), (c.TH, c.Tq))[p]
    bands = [(g + c.MLO[w] >= pp + (c.Tkv - 128) - qoff_w[w])
             .astype(np.float32) for w in range(2)]

    # fold LN affines into the weights (see build_kernel docstring)
    g1 = np.asarray(inputs["ln1_g"], np.float64)
    b1v = np.asarray(inputs["ln1_b"], np.float64)
    g2 = np.asarray(inputs["ln2_g"], np.float64)
    b2v = np.asarray(inputs["ln2_b"], np.float64)
    fq = flat_w(inputs["Wq"]).astype(np.float64)
    fk = flat_w(inputs["Wk"]).astype(np.float64)
    fv = flat_w(inputs["Wv"]).astype(np.float64)
    wp = np.asarray(inputs["Wp"], np.float64)
    w1 = np.asarray(inputs["W1"], np.float64)
    qb = b1v @ fq
    vb = b1v @ fv
    bp_f = np.asarray(inputs["bp"], np.float64) + vb @ wp
    b1_f = np.asarray(inputs["b1"], np.float64) + b2v @ w1

    vl = [qb.astype(np.float32).reshape(-1, 128).T,
          bp_f.astype(np.float32).reshape(-1, 128).T,
          b1_f.astype(np.float32).reshape(-1, 128).T,
          np.asarray(inputs["b2"], np.float32).reshape(-1, 128).T]
    vecs = np.ascontiguousarray(np.concatenate(vl, axis=1))
    return {
        "xT": xT, "xqT": xqT,
        "xqTb": np.ascontiguousarray(xb[qidx].T).astype(ml_dtypes.bfloat16),
        "wq": bf(fq * g1[:, None]), "wk": bf(fk * g1[:, None]),
        "wv": bf(fv * g1[:, None]),
        "wp": bf(wp), "w1": bf(w1 * g2[:, None]),
        "w2": bf(inputs["W2"]),
        "vecs": vecs,
        "maskband0": bf(bands[0]),
        "maskband1": bf(bands[1]),
    }


def host_gather(cfg, results, B):
    c = cfg
    out = np.empty((B, c.Tkv, c.C), np.float32)
    for core in range(2 * B):
        b, p = core // 2, core % 2
        out[b, zigzag_tokens(c, p), :] = \
            np.asarray(results[core]["outT"]).astype(np.float32).T
    return out


_CACHE = {}


def _get_compiled(n_cores=8, ln_affine=True):
    key = "nc"
    if key not in _CACHE:
        cfg = Cfg(C=1024, H=16, D=64, Tkv=2048)
        nc = bacc.Bacc("TRN2", target_bir_lowering=False, debug=False,
                       num_devices=n_cores)
        build_kernel(nc, cfg)
        nc.compile()
        _CACHE[key] = (nc, cfg)
    return _CACHE[key]


def kernel(**inputs):
    """Full transformer block on 8 NeuronCores. Takes the full unsharded
    inputs (as in reference.setup_inputs) and returns the full [4, 2048,
    1024] float32 output."""
    nc, cfg = _get_compiled(8)
    x = np.asarray(inputs["x"])
    B = x.shape[0]
    n_cores = 2 * B
    in_maps = [host_prepare(cfg, inputs, core) for core in range(n_cores)]
    res = run_bass_kernel_spmd(nc, in_maps, core_ids=list(range(n_cores)))
    return host_gather(cfg, res.results, B)
